# revision 13
# baseline (speedup 1.0000x reference)
"""Distributed Trainium2 kernel for nn_ContrastiveLoss (SimCLR InfoNCE loss).

fp8e4 DoubleRow + symmetry, column-progressive phases.

C = zhat zhat^T is symmetric. Rows are rolled per core so core c owns
slab c; block distance d = (col_block - c) mod 8. Each core computes its
8 m-tiles (128 rows each) against local columns 0..5119 in four
column-progressive phases so the first matmul only needs the tail 256
columns of the first 1024-column block:

  A (cols    0:1024): d0 staircase [m*128, 1024), scheduled m=7..0 and
     fed by four 256-col DMA chunks loaded high-to-low — the diag tile
     is computed fully (row-sums; self-diag extracted on-device), the
     strict-upper strip is also column-summed.
  B (cols 1024:2048): nt tiles 2,3 (full, row+col sums).
  C/C2 (cols 2048:4096): nt tiles 4,5 and 6,7 (1024-wide groups so the
     PSUM pool stays 4-deep and PE never stalls on a long exp).
  D (cols 4096:5120): d4 staircase; its diag tile holds the positive
     pairs. The final groups are small and all outputs flush right
     after them.

Every unordered pair is computed exactly once. Row sums accumulate via
ACT accum_out; column partials via DVE adds into a bf16 colacc laid out
as 5 regions of 1024 local columns (region d -> slab (c+d) mod 8).
Diagonal values are extracted on-device (identity mask + row reduce)
into the combined [128,56] acc tile. The host gathers row/column
partials + diag extracts and finishes in fp64.

PE: 33792 output columns x 4 K-passes per core = 135168 PE cycles
~= 56.3 us at 2.4 GHz — the fp8 DoubleRow roofline.
"""

import numpy as np

N, D = 8192, 1024
NCORES = 8
ROWS = N // NCORES
MT = ROWS // 128
KK = 4
NPH = 5                  # 1024-column phases; local cols used: 0..5119
NLOC = NPH * 1024
SCALE = 256.0
EFF = 10.0 / (SCALE * SCALE)


def _import_concourse():
    import sys
    try:
        import concourse.bass  # noqa: F401
    except ImportError:
        for p in ("/root/.axon_site/_ro/trn_rl_repo", "/opt/trn_rl_repo"):
            if p not in sys.path:
                sys.path.insert(0, p)
        import concourse.bass  # noqa: F401


def _a_chunks(m):
    """Phase-A staircase [m*128, 1024): split at 256-col data-chunk
    boundaries AND at psum-bank boundaries (psum offset = col - m*128,
    banks of 512)."""
    lo = m * 128
    cuts = {lo, 1024}
    for c in range(1, 4):
        if lo < c * 256 < 1024:
            cuts.add(c * 256)
    for b in (512, 1024, 1536):
        if lo < lo + b < 1024:
            cuts.add(lo + b)
    cuts = sorted(cuts)
    return [(a - lo, a, b - a) for a, b in zip(cuts, cuts[1:])]


def _groups():
    """(phase, m, acc_col, chunks, strip, diag) in schedule order.
    chunks: list of (psum_off, src, col_off, width) where src is
    ('zq0', chunk_idx) for phase-A cols or ('zq', ph_idx); strip:
    (colacc_lo, esc_lo, width) or None; diag: dext column or None."""
    out = []
    for m in range(MT - 1, -1, -1):  # A: d0 staircase, m=7 first
        w = 1024 - m * 128
        ch = [(off, ("zq0", a // 256), a % 256, cw)
              for off, a, cw in _a_chunks(m)]
        strip = (m * 128 + 128, 128, w - 128) if w > 128 else None
        out.append(("A", m, m, ch, strip, 40 + m))
    for m in range(MT):              # B: nt tiles 2,3
        ch = [(0, ("zq", 0), 0, 512), (512, ("zq", 0), 512, 512)]
        out.append(("B", m, 8 + m, ch, (1024, 0, 1024), None))
    def d_group(m):
        w = 1024 - m * 128
        ch = [(0, ("zq", 3), m * 128, min(512, w))]
        if w > 512:
            ch.append((512, ("zq", 3), m * 128 + 512, w - 512))
        strip = (4096 + m * 128 + 128, 128, w - 128) if w > 128 else None
        return ("D", m, 32 + m, ch, strip, 48 + m)

    for m in range(MT):              # C: nt tiles 4,5 then 6,7
        ch = [(0, ("zq", 1), 0, 512), (512, ("zq", 1), 512, 512)]
        out.append(("C", m, 16 + m, ch, (2048, 0, 1024), None))
        ch = [(0, ("zq", 2), 0, 512), (512, ("zq", 2), 512, 512)]
        out.append(("C2", m, 24 + m, ch, (3072, 0, 1024), None))
    for m in range(MT):              # D: d4 staircase (pos pairs on diag)
        out.append(d_group(m))       # m=7 last: tiny group, outputs
    return out                       # already flushing underneath it


def build_program():
    _import_concourse()
    import concourse.mybir as mybir
    import concourse.tile as tile
    from concourse import bacc
    from concourse.masks import make_identity

    f32 = mybir.dt.float32
    bf16 = mybir.dt.bfloat16
    f8 = mybir.dt.float8e4
    Act = mybir.ActivationFunctionType
    Alu = mybir.AluOpType
    DR = mybir.MatmulPerfMode.DoubleRow

    nc = bacc.Bacc()
    # phase-0 block, stored as 4 column chunks in REVERSED order
    # (cols 768:1024 first) so phase-A m=7.. can start earliest
    zq0 = nc.declare_dram_parameter("zq0", [128, 4 * KK * 2 * 256], f8,
                                    isOutput=False)
    zq = nc.declare_dram_parameter("zq", [128, 4 * KK * 2 * 1024], f8,
                                   isOutput=False)
    accd_d = nc.declare_dram_parameter("accd", [128, 56], f32, isOutput=True)
    colacc_d = nc.declare_dram_parameter("colacc", [128, NLOC], bf16,
                                         isOutput=True)

    zq0_d = zq0.rearrange("p (c kk i n) -> p c kk i n", c=4, kk=KK, i=2)
    zq_d = zq.rearrange("p (ph kk i n) -> p ph kk i n", ph=4, kk=KK, i=2)

    with tile.TileContext(nc) as tc:
        with (
            tc.tile_pool(name="consts", bufs=1) as consts,
            tc.tile_pool(name="zqp", bufs=1) as zqp,
            tc.tile_pool(name="psump", bufs=4, space="PSUM") as psump,
            tc.tile_pool(name="escp", bufs=3) as escp,
            tc.tile_pool(name="smallp", bufs=2) as smallp,
            tc.tile_pool(name="accp", bufs=1) as accp,
        ):
            # warm-up input tile: gpsimd-only init so the PE's first
            # LDWEIGHTS isn't gated behind the ACT exp-table load
            warm8 = consts.tile([128, 2, 512], f8)
            nc.gpsimd.memset(warm8, 0)

            # input DMA, earliest-needed first; 2KB descriptors
            # (max_dma_last_dim) — ~2x the transfer rate of 8KB ones
            zq0_t = zqp.tile([128, 4, KK, 2, 256], f8)
            zq_t = zqp.tile([128, 4, KK, 2, 1024], f8)
            for c in range(4):       # stored high-to-low: chunk 0 = cols 768:1024
                nc.sync.dma_start(out=zq0_t[:, c], in_=zq0_d[:, c])
            for ph in range(4):
                nc.sync.dma_start(out=zq_t[:, ph], in_=zq_d[:, ph],
                                  max_dma_last_dim=2048)

            ident = consts.tile([128, 128], bf16)
            make_identity(nc, ident)
            # ACT exp-table warm-up during the input-DMA window
            actwarm = consts.tile([128, 1], f32)
            nc.scalar.activation(
                out=actwarm, in_=ident[:, :1], func=Act.Exp, scale=0.001
            )

            colacc = accp.tile([128, NLOC], bf16)
            # region 0 on DVE (needed first), the rest on gpsimd
            nc.vector.memset(colacc[:, :1024], 0)
            nc.gpsimd.memset(colacc[:, 1024:], 0)
            accd = accp.tile([128, 56], f32)
            acc = accd[:, :40]
            dext = accd[:, 40:56]

            # PE warm-up while the first input chunk lands: HAM starts
            # at 1.2 GHz; stay busy until real data arrives (~2.4us) —
            # a PE idle gap here resets the clock-ramp timer
            for _ in range(5):
                warmps = psump.tile([128, 1024], f32, name="ps")
                nc.tensor.matmul(
                    warmps[:, :512],
                    lhsT=warm8[:, :, :128],
                    rhs=warm8,
                    start=True,
                    stop=True,
                    perf_mode=DR,
                )

            def rhs_for(src, kk, a, cw):
                kind, idx = src
                if kind == "zq0":
                    # chunk storage is reversed: chunk_idx c holds
                    # cols (3-c)*256 ... so col a in chunk (3 - a//256)
                    return zq0_t[:, 3 - idx, kk, :, a:a + cw]
                return zq_t[:, idx, kk, :, a:a + cw]

            def lhsT_for(m, kk):
                # cols m*128:(m+1)*128 live in zq0 chunk 3 - m//2
                c = 3 - (m * 128) // 256
                a = (m * 128) % 256
                return zq0_t[:, c, kk, :, a:a + 128]

            for ph, m, ai, chunks, strip, di in _groups():
                gw = max(off + cw for off, _, _, cw in chunks)
                ps = psump.tile([128, 1024], f32, name="ps")
                # a start=True matmul zeroes its whole 2KB PSUM bank, so
                # pieces sharing a bank form ONE accumulation group:
                # start on the bank's first piece (kk=0), stop on its
                # last (kk=KK-1)
                first_in_bank, last_in_bank = {}, {}
                for i, (off, _, _, cw) in enumerate(chunks):
                    b = off // 512
                    first_in_bank.setdefault(b, i)
                    last_in_bank[b] = i
                firsts = set(first_in_bank.values())
                lasts = set(last_in_bank.values())
                for kk in range(KK):
                    lhsT = lhsT_for(m, kk)
                    for i, (off, src, ca, cw) in enumerate(chunks):
                        nc.tensor.matmul(
                            ps[:, off:off + cw],
                            lhsT=lhsT,
                            rhs=rhs_for(src, kk, ca, cw),
                            start=(kk == 0 and i in firsts),
                            stop=(kk == KK - 1 and i in lasts),
                            perf_mode=DR,
                        )
                esc = escp.tile([128, 1024], bf16)
                nc.scalar.activation(
                    out=esc[:, :gw],
                    in_=ps[:, :gw],
                    func=Act.Exp,
                    scale=EFF,
                    accum_out=acc[:, ai:ai + 1],
                )
                if di is not None:
                    # diag of the first 128-wide tile: identity-mask the
                    # exp'd tile, then row-reduce
                    tmp = smallp.tile([128, 128], bf16)
                    nc.vector.tensor_tensor(
                        out=tmp, in0=esc[:, :128], in1=ident, op=Alu.mult
                    )
                    nc.vector.reduce_sum(
                        out=accd[:, di:di + 1], in_=tmp,
                        axis=mybir.AxisListType.X,
                    )
                if strip is not None:
                    lo, elo, w = strip
                    dst = colacc[:, lo:lo + w]
                    nc.vector.tensor_add(out=dst, in0=dst,
                                         in1=esc[:, elo:elo + w])
                # flush column-partial regions as their last writer runs
                if (ph, m) == ("A", 0):
                    nc.sync.dma_start(out=colacc_d[:, :1024],
                                      in_=colacc[:, :1024])
                if (ph, m) == ("B", 7):
                    nc.sync.dma_start(out=colacc_d[:, 1024:2048],
                                      in_=colacc[:, 1024:2048])
                if (ph, m) == ("C2", 7):
                    nc.sync.dma_start(out=colacc_d[:, 2048:4096],
                                      in_=colacc[:, 2048:4096])
                # region 4 in two pieces: cols < 4992 are final after
                # D5, the last 128 after D6 — both flushes overlap the
                # remaining D groups' compute
                if (ph, m) == ("D", 5):
                    nc.sync.dma_start(out=colacc_d[:, 4096:4992],
                                      in_=colacc[:, 4096:4992])
                if (ph, m) == ("D", 6):
                    nc.sync.dma_start(out=colacc_d[:, 4992:],
                                      in_=colacc[:, 4992:])
                if (ph, m) == ("D", 7):
                    nc.scalar.dma_start(out=accd_d[:, :], in_=accd)
    nc.finalize()
    return nc


def make_in_maps(z: np.ndarray) -> list[dict]:
    import ml_dtypes

    z = np.ascontiguousarray(np.asarray(z, dtype=np.float32))
    norms = np.sqrt((z.astype(np.float64) ** 2).sum(axis=-1))
    zn = (z / norms[:, None]).astype(np.float32)
    q = (zn * np.float32(SCALE)).astype(ml_dtypes.float8_e4m3)  # [N, D]
    qt = np.ascontiguousarray(q.T)  # [D, N]
    qr = qt.reshape(KK, 2, 128, N).transpose(2, 0, 1, 3)  # [p, kk, i, n]
    in_maps = []
    for c in range(NCORES):
        s = c * ROWS
        zc = np.concatenate([qr[..., s:], qr[..., :s]], axis=-1) if s else qr
        zc = zc[..., :NLOC]                       # [p, kk, i, 5120]
        # phase 0 (cols 0:1024) as 4 column chunks, REVERSED order
        p0 = zc[..., :1024].reshape(128, KK, 2, 4, 256)
        p0 = p0[:, :, :, ::-1].transpose(0, 3, 1, 2, 4)  # [p, c, kk, i, 256]
        # phases 1..4 as 1024-col blocks
        pr = zc[..., 1024:].reshape(128, KK, 2, 4, 1024)
        pr = pr.transpose(0, 3, 1, 2, 4)                 # [p, ph, kk, i, 1024]
        in_maps.append({
            "zq0": np.ascontiguousarray(p0.reshape(128, -1)),
            "zq": np.ascontiguousarray(pr.reshape(128, -1)),
        })
    return in_maps


def assemble(results: list[dict]) -> np.ndarray:
    S = np.zeros(N, np.float64)
    pvals = np.zeros(N, np.float64)
    for c, r in enumerate(results):
        accd = np.asarray(r["accd"], np.float64)      # [128, 56]
        acc, dext = accd[:, :40], accd[:, 40:56]
        colacc = np.asarray(r["colacc"], np.float64)  # [128, 5120]
        base = c * ROWS
        for m in range(MT):
            rows = base + m * 128 + np.arange(128)
            tot = (acc[:, m] + acc[:, 8 + m] + acc[:, 16 + m]
                   + acc[:, 24 + m] + acc[:, 32 + m])
            # self term was counted once (row-sums of the d0 diag tile);
            # dext[:, m] holds exp(EFF*selfdot) directly
            S[rows] += tot - dext[:, m]
            pvals[rows] = dext[:, 8 + m]              # exp(EFF*posdot)
        csum = colacc.sum(axis=0)                     # [5120]
        for d in range(NPH):
            dest = ((c + d) % NCORES) * ROWS + np.arange(1024)
            S[dest] += csum[d * 1024:(d + 1) * 1024]
    nll = np.log(S) - np.log(pvals)
    return np.float32(nll.mean())


def kernel(z: np.ndarray) -> np.ndarray:
    _import_concourse()
    from concourse.bass_utils import run_bass_kernel_spmd

    nc = build_program()
    in_maps = make_in_maps(z)
    res = run_bass_kernel_spmd(nc, in_maps, core_ids=list(range(NCORES)))
    return assemble(res.results)


# revision 14
# speedup vs baseline: 1.0102x; 1.0102x over previous
"""Distributed Trainium2 kernel for nn_ContrastiveLoss (SimCLR InfoNCE loss).

fp8e4 DoubleRow + symmetry, column-progressive phases.

C = zhat zhat^T is symmetric. Rows are rolled per core so core c owns
slab c; block distance d = (col_block - c) mod 8. Each core computes its
8 m-tiles (128 rows each) against local columns 0..5119 in four
column-progressive phases so the first matmul only needs the tail 256
columns of the first 1024-column block:

  A (cols    0:1024): d0 staircase [m*128, 1024), scheduled m=7..0 and
     fed by four 256-col DMA chunks loaded high-to-low — the diag tile
     is computed fully (row-sums; self-diag extracted on-device), the
     strict-upper strip is also column-summed.
  B (cols 1024:2048): nt tiles 2,3 (full, row+col sums).
  C/C2 (cols 2048:4096): nt tiles 4,5 and 6,7 (1024-wide groups so the
     PSUM pool stays 4-deep and PE never stalls on a long exp).
  D (cols 4096:5120): d4 staircase; its diag tile holds the positive
     pairs. The final groups are small and all outputs flush right
     after them.

Every unordered pair is computed exactly once. Row sums accumulate via
ACT accum_out; column partials via DVE adds into a bf16 colacc laid out
as 5 regions of 1024 local columns (region d -> slab (c+d) mod 8).
Diagonal values are extracted on-device (identity mask + row reduce)
into the combined [128,56] acc tile. The host gathers row/column
partials + diag extracts and finishes in fp64.

PE: 33792 output columns x 4 K-passes per core = 135168 PE cycles
~= 56.3 us at 2.4 GHz — the fp8 DoubleRow roofline.
"""

import numpy as np

N, D = 8192, 1024
NCORES = 8
ROWS = N // NCORES
MT = ROWS // 128
KK = 4
NPH = 5                  # 1024-column phases; local cols used: 0..5119
NLOC = NPH * 1024
SCALE = 256.0
EFF = 10.0 / (SCALE * SCALE)


def _import_concourse():
    import sys
    try:
        import concourse.bass  # noqa: F401
    except ImportError:
        for p in ("/root/.axon_site/_ro/trn_rl_repo", "/opt/trn_rl_repo"):
            if p not in sys.path:
                sys.path.insert(0, p)
        import concourse.bass  # noqa: F401


def _a_chunks(m):
    """Phase-A staircase [m*128, 1024): split at 256-col data-chunk
    boundaries AND at psum-bank boundaries (psum offset = col - m*128,
    banks of 512)."""
    lo = m * 128
    cuts = {lo, 1024}
    for c in (256, 512, 768, 896):
        if lo < c < 1024:
            cuts.add(c)
    for b in (512, 1024, 1536):
        if lo < lo + b < 1024:
            cuts.add(lo + b)
    cuts = sorted(cuts)
    return [(a - lo, a, b - a) for a, b in zip(cuts, cuts[1:])]


def _groups():
    """(phase, m, acc_col, chunks, strip, diag) in schedule order.
    chunks: list of (psum_off, src, col_off, width) where src is
    ('zq0', chunk_idx) for phase-A cols or ('zq', ph_idx); strip:
    (colacc_lo, esc_lo, width) or None; diag: dext column or None."""
    out = []
    for m in range(MT - 1, -1, -1):  # A: d0 staircase, m=7 first
        w = 1024 - m * 128
        def src_of(a):
            # (tile, chunk_idx, col_base): 128-col chunks a0/a1 first
            if a >= 896:
                return ("a", 0, 896)
            if a >= 768:
                return ("a", 1, 768)
            return ("b", (767 - a) // 256, (a // 256) * 256)
        ch = []
        for off, a, cw in _a_chunks(m):
            t, ci, base = src_of(a)
            ch.append((off, (t, ci), a - base, cw))
        strip = (m * 128 + 128, 128, w - 128) if w > 128 else None
        out.append(("A", m, m, ch, strip, 40 + m))
    for m in range(MT):              # B: nt tiles 2,3
        ch = [(0, ("zq", 0), 0, 512), (512, ("zq", 0), 512, 512)]
        out.append(("B", m, 8 + m, ch, (1024, 0, 1024), None))
    def d_group(m):
        w = 1024 - m * 128
        ch = [(0, ("zq", 3), m * 128, min(512, w))]
        if w > 512:
            ch.append((512, ("zq", 3), m * 128 + 512, w - 512))
        strip = (4096 + m * 128 + 128, 128, w - 128) if w > 128 else None
        return ("D", m, 32 + m, ch, strip, 48 + m)

    for m in range(MT):              # C: nt tiles 4,5 then 6,7
        ch = [(0, ("zq", 1), 0, 512), (512, ("zq", 1), 512, 512)]
        out.append(("C", m, 16 + m, ch, (2048, 0, 1024), None))
        ch = [(0, ("zq", 2), 0, 512), (512, ("zq", 2), 512, 512)]
        out.append(("C2", m, 24 + m, ch, (3072, 0, 1024), None))
    for m in range(MT):              # D: d4 staircase (pos pairs on diag)
        out.append(d_group(m))       # m=7 last: tiny group, outputs
    return out                       # already flushing underneath it


def build_program():
    _import_concourse()
    import concourse.mybir as mybir
    import concourse.tile as tile
    from concourse import bacc
    from concourse.masks import make_identity

    f32 = mybir.dt.float32
    bf16 = mybir.dt.bfloat16
    f8 = mybir.dt.float8e4
    Act = mybir.ActivationFunctionType
    Alu = mybir.AluOpType
    DR = mybir.MatmulPerfMode.DoubleRow

    nc = bacc.Bacc()
    # phase-0 block, stored as 4 column chunks in REVERSED order
    # (cols 768:1024 first) so phase-A m=7.. can start earliest
    zq0a = nc.declare_dram_parameter("zq0a", [128, 2 * KK * 2 * 128], f8,
                                     isOutput=False)
    zq0b = nc.declare_dram_parameter("zq0b", [128, 3 * KK * 2 * 256], f8,
                                     isOutput=False)
    zq = nc.declare_dram_parameter("zq", [128, 4 * KK * 2 * 1024], f8,
                                   isOutput=False)
    accd_d = nc.declare_dram_parameter("accd", [128, 56], f32, isOutput=True)
    colacc_d = nc.declare_dram_parameter("colacc", [128, NLOC], bf16,
                                         isOutput=True)

    zq0a_d = zq0a.rearrange("p (c kk i n) -> p c kk i n", c=2, kk=KK, i=2)
    zq0b_d = zq0b.rearrange("p (c kk i n) -> p c kk i n", c=3, kk=KK, i=2)
    zq_d = zq.rearrange("p (ph kk i n) -> p ph kk i n", ph=4, kk=KK, i=2)

    with tile.TileContext(nc) as tc:
        with (
            tc.tile_pool(name="consts", bufs=1) as consts,
            tc.tile_pool(name="zqp", bufs=1) as zqp,
            tc.tile_pool(name="psump", bufs=4, space="PSUM") as psump,
            tc.tile_pool(name="escp", bufs=3) as escp,
            tc.tile_pool(name="smallp", bufs=2) as smallp,
            tc.tile_pool(name="accp", bufs=1) as accp,
        ):
            # warm-up input tile: gpsimd-only init so the PE's first
            # LDWEIGHTS isn't gated behind the ACT exp-table load
            warm8 = consts.tile([128, 2, 512], f8)
            nc.gpsimd.memset(warm8, 0)

            # input DMA, earliest-needed first; 2KB descriptors
            # (max_dma_last_dim) — ~2x the transfer rate of 8KB ones
            zq0a_t = zqp.tile([128, 2, KK, 2, 128], f8)
            zq0b_t = zqp.tile([128, 3, KK, 2, 256], f8)
            zq_t = zqp.tile([128, 4, KK, 2, 1024], f8)
            # cols 896:1024 via sync and 768:896 via the ACT hwdge queue
            # in PARALLEL, then 512:768, 256:512, 0:256 high-to-low
            nc.sync.dma_start(out=zq0a_t[:, 0], in_=zq0a_d[:, 0])
            nc.scalar.dma_start(out=zq0a_t[:, 1], in_=zq0a_d[:, 1])
            for c in range(3):
                nc.sync.dma_start(out=zq0b_t[:, c], in_=zq0b_d[:, c])
            for ph in range(4):
                nc.sync.dma_start(out=zq_t[:, ph], in_=zq_d[:, ph],
                                  max_dma_last_dim=2048)

            ident = consts.tile([128, 128], bf16)
            make_identity(nc, ident)
            # ACT exp-table warm-up during the input-DMA window
            actwarm = consts.tile([128, 1], f32)
            nc.scalar.activation(
                out=actwarm, in_=ident[:, :1], func=Act.Exp, scale=0.001
            )

            colacc = accp.tile([128, NLOC], bf16)
            # region 0 on DVE (needed first), the rest on gpsimd
            nc.vector.memset(colacc[:, :1024], 0)
            nc.gpsimd.memset(colacc[:, 1024:], 0)
            accd = accp.tile([128, 56], f32)
            acc = accd[:, :40]
            dext = accd[:, 40:56]

            # PE warm-up while the first input chunk lands: HAM starts
            # at 1.2 GHz; stay busy until real data arrives (~2.4us) —
            # a PE idle gap here resets the clock-ramp timer
            for _ in range(5):
                warmps = psump.tile([128, 1024], f32, name="ps")
                nc.tensor.matmul(
                    warmps[:, :512],
                    lhsT=warm8[:, :, :128],
                    rhs=warm8,
                    start=True,
                    stop=True,
                    perf_mode=DR,
                )

            def rhs_for(src, kk, a, cw):
                kind, idx = src
                if kind == "a":
                    return zq0a_t[:, idx, kk, :, a:a + cw]
                if kind == "b":
                    return zq0b_t[:, idx, kk, :, a:a + cw]
                return zq_t[:, idx, kk, :, a:a + cw]

            _LHS = {7: ("a", 0, 0), 6: ("a", 1, 0), 5: ("b", 0, 128),
                    4: ("b", 0, 0), 3: ("b", 1, 128), 2: ("b", 1, 0),
                    1: ("b", 2, 128), 0: ("b", 2, 0)}

            def lhsT_for(m, kk):
                t, c, a = _LHS[m]
                tile = zq0a_t if t == "a" else zq0b_t
                return tile[:, c, kk, :, a:a + 128]

            for ph, m, ai, chunks, strip, di in _groups():
                gw = max(off + cw for off, _, _, cw in chunks)
                ps = psump.tile([128, 1024], f32, name="ps")
                # a start=True matmul zeroes its whole 2KB PSUM bank, so
                # pieces sharing a bank form ONE accumulation group:
                # start on the bank's first piece (kk=0), stop on its
                # last (kk=KK-1)
                first_in_bank, last_in_bank = {}, {}
                for i, (off, _, _, cw) in enumerate(chunks):
                    b = off // 512
                    first_in_bank.setdefault(b, i)
                    last_in_bank[b] = i
                firsts = set(first_in_bank.values())
                lasts = set(last_in_bank.values())
                for kk in range(KK):
                    lhsT = lhsT_for(m, kk)
                    for i, (off, src, ca, cw) in enumerate(chunks):
                        nc.tensor.matmul(
                            ps[:, off:off + cw],
                            lhsT=lhsT,
                            rhs=rhs_for(src, kk, ca, cw),
                            start=(kk == 0 and i in firsts),
                            stop=(kk == KK - 1 and i in lasts),
                            perf_mode=DR,
                        )
                esc = escp.tile([128, 1024], bf16)
                nc.scalar.activation(
                    out=esc[:, :gw],
                    in_=ps[:, :gw],
                    func=Act.Exp,
                    scale=EFF,
                    accum_out=acc[:, ai:ai + 1],
                )
                if di is not None:
                    # diag of the first 128-wide tile: identity-mask the
                    # exp'd tile, then row-reduce
                    tmp = smallp.tile([128, 128], bf16)
                    nc.vector.tensor_tensor(
                        out=tmp, in0=esc[:, :128], in1=ident, op=Alu.mult
                    )
                    nc.vector.reduce_sum(
                        out=accd[:, di:di + 1], in_=tmp,
                        axis=mybir.AxisListType.X,
                    )
                if strip is not None:
                    lo, elo, w = strip
                    dst = colacc[:, lo:lo + w]
                    nc.vector.tensor_add(out=dst, in0=dst,
                                         in1=esc[:, elo:elo + w])
                # flush column-partial regions as their last writer runs
                if (ph, m) == ("A", 0):
                    nc.sync.dma_start(out=colacc_d[:, :1024],
                                      in_=colacc[:, :1024])
                if (ph, m) == ("B", 7):
                    nc.sync.dma_start(out=colacc_d[:, 1024:2048],
                                      in_=colacc[:, 1024:2048])
                if (ph, m) == ("C2", 7):
                    nc.sync.dma_start(out=colacc_d[:, 2048:4096],
                                      in_=colacc[:, 2048:4096])
                # region 4 in two pieces: cols < 4992 are final after
                # D5, the last 128 after D6 — both flushes overlap the
                # remaining D groups' compute
                if (ph, m) == ("D", 5):
                    nc.sync.dma_start(out=colacc_d[:, 4096:4992],
                                      in_=colacc[:, 4096:4992])
                if (ph, m) == ("D", 6):
                    nc.sync.dma_start(out=colacc_d[:, 4992:],
                                      in_=colacc[:, 4992:])
                if (ph, m) == ("D", 7):
                    nc.scalar.dma_start(out=accd_d[:, :], in_=accd)
    nc.finalize()
    return nc


def make_in_maps(z: np.ndarray) -> list[dict]:
    import ml_dtypes

    z = np.ascontiguousarray(np.asarray(z, dtype=np.float32))
    norms = np.sqrt((z.astype(np.float64) ** 2).sum(axis=-1))
    zn = (z / norms[:, None]).astype(np.float32)
    q = (zn * np.float32(SCALE)).astype(ml_dtypes.float8_e4m3)  # [N, D]
    qt = np.ascontiguousarray(q.T)  # [D, N]
    qr = qt.reshape(KK, 2, 128, N).transpose(2, 0, 1, 3)  # [p, kk, i, n]
    in_maps = []
    for c in range(NCORES):
        s = c * ROWS
        zc = np.concatenate([qr[..., s:], qr[..., :s]], axis=-1) if s else qr
        zc = zc[..., :NLOC]                       # [p, kk, i, 5120]
        # phase 0 (cols 0:1024): two 128-col chunks (896:1024, 768:896)
        # then three 256-col chunks high-to-low
        p0 = zc[..., :1024]                              # [p, kk, i, 1024]
        pa = np.stack([p0[..., 896:1024], p0[..., 768:896]], axis=1)
        pb = np.stack([p0[..., 512:768], p0[..., 256:512],
                       p0[..., 0:256]], axis=1)          # [p, 3, kk, i, 256]
        # phases 1..4 as 1024-col blocks
        pr = zc[..., 1024:].reshape(128, KK, 2, 4, 1024)
        pr = pr.transpose(0, 3, 1, 2, 4)                 # [p, ph, kk, i, 1024]
        in_maps.append({
            "zq0a": np.ascontiguousarray(pa.reshape(128, -1)),
            "zq0b": np.ascontiguousarray(pb.reshape(128, -1)),
            "zq": np.ascontiguousarray(pr.reshape(128, -1)),
        })
    return in_maps


def assemble(results: list[dict]) -> np.ndarray:
    S = np.zeros(N, np.float64)
    pvals = np.zeros(N, np.float64)
    for c, r in enumerate(results):
        accd = np.asarray(r["accd"], np.float64)      # [128, 56]
        acc, dext = accd[:, :40], accd[:, 40:56]
        colacc = np.asarray(r["colacc"], np.float64)  # [128, 5120]
        base = c * ROWS
        for m in range(MT):
            rows = base + m * 128 + np.arange(128)
            tot = (acc[:, m] + acc[:, 8 + m] + acc[:, 16 + m]
                   + acc[:, 24 + m] + acc[:, 32 + m])
            # self term was counted once (row-sums of the d0 diag tile);
            # dext[:, m] holds exp(EFF*selfdot) directly
            S[rows] += tot - dext[:, m]
            pvals[rows] = dext[:, 8 + m]              # exp(EFF*posdot)
        csum = colacc.sum(axis=0)                     # [5120]
        for d in range(NPH):
            dest = ((c + d) % NCORES) * ROWS + np.arange(1024)
            S[dest] += csum[d * 1024:(d + 1) * 1024]
    nll = np.log(S) - np.log(pvals)
    return np.float32(nll.mean())


def kernel(z: np.ndarray) -> np.ndarray:
    _import_concourse()
    from concourse.bass_utils import run_bass_kernel_spmd

    nc = build_program()
    in_maps = make_in_maps(z)
    res = run_bass_kernel_spmd(nc, in_maps, core_ids=list(range(NCORES)))
    return assemble(res.results)
